# revision 18
# baseline (speedup 1.0000x reference)
"""Trainium2 Bass kernel for the ATripletMarginLossOHNMDM loss.

Per row i of an (B, B) input:
  sim_p      = input[i, i]
  masked     = where(target[i]==0, input[i], -1e9)
  sim_n[0:3] = top-3 values of masked          (hard negatives)
  d          = clip(|sim_p - sim_n|, 0.1, 0.3)
  loss       = relu(sim_n - sim_p + d)
  s          = where(loss>0, sim_n, -50)
  w          = softmax(s / 0.1)      (with max-subtraction, as jax.nn.softmax)
  out        = mean over (B, 3) of loss * w

Sharded by rows across 8 NeuronCores (1024 rows each). Per core, per
128-row tile:
  - DMA the input rows (f32) and the target rows. The target holds only
    0/1, so the host passes `target.view(int8)[:, ::4]` — a strided byte
    view of the original int32 buffer (pure data movement, no
    arithmetic) — which cuts target DMA traffic 4x.
  - one DVE scalar_tensor_tensor applies the mask in place:
        m = (target * -1e9) + input
  - one DVE Max8 instruction returns the top-8 per row -> top-3,
    collected into a [128, n_tiles, 8] buffer
A single vectorized epilogue then computes the margin/softmax math for
all tiles at once on [128, n_tiles, 3], and the per-(partition, tile)
partial sums are DMA'd out as [128, n_tiles]. The final mean over the
8 * 128 * n_tiles partials is computed on host.
"""

import numpy as np

import concourse.bacc as bacc
import concourse.mybir as mybir
import concourse.tile as tile
from concourse.bass_utils import run_bass_kernel_spmd

_B = 8192          # full problem size (rows == cols)
_NCORES = 8
_P = 128           # SBUF partitions
_K = 3
_BIG_NEG = -1.0e9  # mask fill; far below any real similarity
_NEG_FILL = -50.0  # reference's softmax mask fill (must match exactly)
_INV_TAU = 10.0    # 1 / 0.1
_TGT_INT8 = True   # pass target as int8 byte view (0/1 values, lossless)
# Columns of the mask pass offloaded to GPSIMD (as two tensor_tensor ops —
# the fused scalar_tensor_tensor opcode is not legal on Pool). Pool ops take
# the SBUF port pair shared with the DVE, so the chunk is sized to run inside
# the DVE Max8 window (Max8 only uses the DVE-dedicated port). 0 disables.
_GP_COLS = 2048


def _build_nc(rows_per_core: int, ncols: int) -> bacc.Bacc:
    n_tiles = rows_per_core // _P
    f32 = mybir.dt.float32
    i32 = mybir.dt.int32
    tdt = mybir.dt.int8 if _TGT_INT8 else i32

    nc = bacc.Bacc()
    inp = nc.dram_tensor("inp", [rows_per_core, ncols], f32,
                         kind="ExternalInput")
    tgt = nc.dram_tensor("tgt", [rows_per_core, ncols], tdt,
                         kind="ExternalInput")
    # diag[p, t] = input diagonal element of local row t*128 + p
    diag = nc.dram_tensor("diag", [_P, n_tiles], f32, kind="ExternalInput")
    out = nc.dram_tensor("out", [_P, n_tiles], f32, kind="ExternalOutput")

    with tile.TileContext(nc) as tc:
        with (
            tc.tile_pool(name="singles", bufs=1) as singles,
            tc.tile_pool(name="io_in", bufs=3) as io_in,
            tc.tile_pool(name="io_tg", bufs=3) as io_tg,
            tc.tile_pool(name="gp", bufs=2) as gp_pool,
            tc.tile_pool(name="small", bufs=1) as small,
        ):
            diag_raw = singles.tile([_P, n_tiles], f32)
            nc.scalar.dma_start(out=diag_raw, in_=diag[:, :])
            diag_sb = singles.tile([_P, n_tiles], f32)
            nc.vector.tensor_copy(out=diag_sb, in_=diag_raw)
            negbig = singles.tile([_P, 1], f32)
            nc.vector.memset(negbig, _BIG_NEG)
            # top-8 per (row, tile), filled by the main loop
            vfin = singles.tile([_P, n_tiles, 8], f32)

            # tile 0 is processed in column chunks (DVE-only) so compute
            # starts as soon as the first chunk lands, instead of waiting
            # for the full 5 MiB tile-0 DMA
            chunk0 = ncols >= 4096 and ncols % 4 == 0
            vcol0 = None
            if chunk0:
                vcol0 = singles.tile([_P, 4, 8], f32)

            for t in range(n_tiles):
                rows = slice(t * _P, (t + 1) * _P)
                in_t = io_in.tile([_P, ncols], f32)
                tg_t = io_tg.tile([_P, ncols], tdt)
                if t == 0 and chunk0:
                    w = ncols // 4
                    for c in range(4):
                        cs = slice(c * w, (c + 1) * w)
                        nc.sync.dma_start(out=in_t[:, cs], in_=inp[rows, cs])
                        nc.scalar.dma_start(out=tg_t[:, cs], in_=tgt[rows, cs])
                    for c in range(4):
                        cs = slice(c * w, (c + 1) * w)
                        nc.vector.scalar_tensor_tensor(
                            out=in_t[:, cs], in0=tg_t[:, cs], scalar=_BIG_NEG,
                            in1=in_t[:, cs],
                            op0=mybir.AluOpType.mult, op1=mybir.AluOpType.add)
                        nc.vector.max(out=vcol0[:, c, :], in_=in_t[:, cs])
                    nc.vector.max(out=vfin[:, 0, :], in_=vcol0[:, :, :])
                    continue
                nc.sync.dma_start(out=in_t, in_=inp[rows, :])
                nc.scalar.dma_start(out=tg_t, in_=tgt[rows, :])
                # masked = (target * -1e9) + input, in place; the column
                # range is split between GPSIMD and DVE (see _GP_COLS),
                # with the GPSIMD share in two sub-chunks to shorten its
                # exclusive holds of the shared SBUF port
                gp_cols = min(_GP_COLS, ncols // 4)
                sp = ncols - gp_cols
                if gp_cols:
                    half = gp_cols // 2
                    for k in range(2):
                        ks = slice(sp + k * half,
                                   sp + (k + 1) * half if k == 0 else ncols)
                        kn = (ks.stop - ks.start)
                        ug = gp_pool.tile([_P, half], f32, tag=f"ug{k}")
                        nc.gpsimd.tensor_tensor(
                            out=ug[:, :kn], in0=tg_t[:, ks],
                            in1=negbig.to_broadcast([_P, kn]),
                            op=mybir.AluOpType.mult)
                        nc.gpsimd.tensor_tensor(
                            out=in_t[:, ks], in0=in_t[:, ks], in1=ug[:, :kn],
                            op=mybir.AluOpType.add)
                nc.vector.scalar_tensor_tensor(
                    out=in_t[:, :sp], in0=tg_t[:, :sp], scalar=_BIG_NEG,
                    in1=in_t[:, :sp],
                    op0=mybir.AluOpType.mult, op1=mybir.AluOpType.add)
                nc.vector.max(out=vfin[:, t, :], in_=in_t)

            # ---- vectorized epilogue over all tiles: [128, n_tiles, 3] ----
            sh = [_P, n_tiles, _K]
            v = small.tile(sh, f32)                    # top-3, descending
            nc.vector.tensor_copy(out=v, in_=vfin[:, :, 0:_K])
            p_b = diag_sb.unsqueeze(-1).to_broadcast(sh)

            x = small.tile(sh, f32)                    # x = sim_n - sim_p
            nc.vector.tensor_tensor(out=x, in0=v, in1=p_b,
                                    op=mybir.AluOpType.subtract)
            # a = clip(|x|, 0.1, 0.3)   (|x| as max(x, -x), bitwise exact)
            negx = small.tile(sh, f32)
            nc.vector.tensor_scalar(out=negx, in0=x, scalar1=-1.0,
                                    scalar2=None, op0=mybir.AluOpType.mult)
            a = small.tile(sh, f32)
            nc.vector.tensor_tensor(out=a, in0=x, in1=negx,
                                    op=mybir.AluOpType.max)
            nc.vector.tensor_scalar(out=a, in0=a, scalar1=0.1, scalar2=0.3,
                                    op0=mybir.AluOpType.max,
                                    op1=mybir.AluOpType.min)
            # loss = relu(x + a); active = (x + a) > 0
            xa = small.tile(sh, f32)
            nc.vector.tensor_tensor(out=xa, in0=x, in1=a,
                                    op=mybir.AluOpType.add)
            l = small.tile(sh, f32)
            nc.vector.tensor_scalar(out=l, in0=xa, scalar1=0.0, scalar2=None,
                                    op0=mybir.AluOpType.max)
            act = small.tile(sh, i32)
            nc.vector.tensor_scalar(out=act, in0=xa, scalar1=0.0, scalar2=None,
                                    op0=mybir.AluOpType.is_gt)
            # s = where(active, v, -50)
            s = small.tile(sh, f32)
            nc.vector.memset(s, _NEG_FILL)
            nc.vector.copy_predicated(out=s, mask=act, data=v)
            # softmax(s / tau) over K, with max-subtraction (matches jax)
            smax = small.tile([_P, n_tiles], f32)
            nc.vector.reduce_max(out=smax, in_=s, axis=mybir.AxisListType.X)
            s2 = small.tile(sh, f32)
            nc.vector.tensor_tensor(out=s2, in0=s,
                                    in1=smax.unsqueeze(-1).to_broadcast(sh),
                                    op=mybir.AluOpType.subtract)
            e = small.tile(sh, f32)
            nc.scalar.activation(out=e, in_=s2,
                                 func=mybir.ActivationFunctionType.Exp,
                                 scale=_INV_TAU)
            z = small.tile([_P, n_tiles], f32)
            nc.vector.reduce_sum(out=z, in_=e, axis=mybir.AxisListType.X)
            r = small.tile([_P, n_tiles], f32)
            nc.vector.reciprocal(out=r, in_=z)
            w = small.tile(sh, f32)
            nc.vector.tensor_tensor(out=w, in0=e,
                                    in1=r.unsqueeze(-1).to_broadcast(sh),
                                    op=mybir.AluOpType.mult)
            lw = small.tile(sh, f32)
            nc.vector.tensor_tensor(out=lw, in0=l, in1=w,
                                    op=mybir.AluOpType.mult)
            out_sb = small.tile([_P, n_tiles], f32)
            nc.vector.reduce_sum(out=out_sb, in_=lw, axis=mybir.AxisListType.X)
            nc.sync.dma_start(out=out[:, :], in_=out_sb)
    nc.compile()
    return nc


def _prepare_in_maps(inp: np.ndarray, tgt: np.ndarray, ncores: int):
    b, ncols = inp.shape
    rows = b // ncores
    n_tiles = rows // _P
    d = np.ascontiguousarray(np.diagonal(inp)).astype(np.float32, copy=False)
    if _TGT_INT8:
        # 0/1 int32 little-endian: byte 0 of each element carries the value
        tgt_v = np.ascontiguousarray(tgt.view(np.int8)[:, ::4])
    else:
        tgt_v = tgt
    in_maps = []
    for c in range(ncores):
        sl = slice(c * rows, (c + 1) * rows)
        diag_c = np.ascontiguousarray(d[sl].reshape(n_tiles, _P).T)
        in_maps.append({
            "inp": np.ascontiguousarray(inp[sl]),
            "tgt": np.ascontiguousarray(tgt_v[sl]),
            "diag": diag_c,
        })
    return in_maps


def kernel(input, target):
    inp = np.asarray(input, dtype=np.float32)
    tgt = np.asarray(target, dtype=np.int32)
    b, ncols = inp.shape

    nc = _build_nc(b // _NCORES, ncols)
    in_maps = _prepare_in_maps(inp, tgt, _NCORES)
    res = run_bass_kernel_spmd(nc, in_maps, list(range(_NCORES)))
    total = 0.0
    for r in res.results:
        total += r["out"].astype(np.float64).sum()
    return np.asarray(total / (b * _K), dtype=np.float32)


if __name__ == "__main__":
    rng = np.random.default_rng(0)
    b = _B
    x = rng.standard_normal((b, b), dtype=np.float32)
    t = rng.integers(0, 2, size=(b, b)).astype(np.int32)
    np.fill_diagonal(t, 1)
    print(kernel(x, t))


# revision 19
# speedup vs baseline: 1.2249x; 1.2249x over previous
"""Trainium2 Bass kernel for the ATripletMarginLossOHNMDM loss.

Per row i of an (B, B) input:
  sim_p      = input[i, i]
  masked     = where(target[i]==0, input[i], -1e9)
  sim_n[0:3] = top-3 values of masked          (hard negatives)
  d          = clip(|sim_p - sim_n|, 0.1, 0.3)
  loss       = relu(sim_n - sim_p + d)
  s          = where(loss>0, sim_n, -50)
  w          = softmax(s / 0.1)      (with max-subtraction, as jax.nn.softmax)
  out        = mean over (B, 3) of loss * w

Sharded by rows across 8 NeuronCores (1024 rows each). Per core, per
128-row tile:
  - DMA the input rows (f32) and the target rows. The target holds only
    0/1, so the host passes `target.view(int8)[:, ::4]` — a strided byte
    view of the original int32 buffer (pure data movement, no
    arithmetic) — which cuts target DMA traffic 4x.
  - one DVE scalar_tensor_tensor applies the mask in place:
        m = (target * -1e9) + input
  - one DVE Max8 instruction returns the top-8 per row -> top-3,
    collected into a [128, n_tiles, 8] buffer
A single vectorized epilogue then computes the margin/softmax math for
all tiles at once on [128, n_tiles, 3], and the per-(partition, tile)
partial sums are DMA'd out as [128, n_tiles]. The final mean over the
8 * 128 * n_tiles partials is computed on host.
"""

import numpy as np

import concourse.bacc as bacc
import concourse.mybir as mybir
import concourse.tile as tile
from concourse.bass_utils import run_bass_kernel_spmd

_B = 8192          # full problem size (rows == cols)
_NCORES = 8
_P = 128           # SBUF partitions
_K = 3
_BIG_NEG = -1.0e9  # mask fill; far below any real similarity
_NEG_FILL = -50.0  # reference's softmax mask fill (must match exactly)
_INV_TAU = 10.0    # 1 / 0.1
_TGT_INT8 = True   # pass target as int8 byte view (0/1 values, lossless)
# Columns of the mask pass offloaded to GPSIMD (as two tensor_tensor ops —
# the fused scalar_tensor_tensor opcode is not legal on Pool). Pool ops take
# the SBUF port pair shared with the DVE, so the chunk is sized to run inside
# the DVE Max8 window (Max8 only uses the DVE-dedicated port). 0 disables.
_GP_COLS = 2048


def _build_nc(rows_per_core: int, ncols: int) -> bacc.Bacc:
    n_tiles = rows_per_core // _P
    f32 = mybir.dt.float32
    i32 = mybir.dt.int32
    tdt = mybir.dt.int8 if _TGT_INT8 else i32

    nc = bacc.Bacc()
    inp = nc.dram_tensor("inp", [rows_per_core, ncols], f32,
                         kind="ExternalInput")
    tgt = nc.dram_tensor("tgt", [rows_per_core, ncols], tdt,
                         kind="ExternalInput")
    # diag[p, t] = input diagonal element of local row t*128 + p
    diag = nc.dram_tensor("diag", [_P, n_tiles], f32, kind="ExternalInput")
    out = nc.dram_tensor("out", [_P, n_tiles], f32, kind="ExternalOutput")

    with tile.TileContext(nc) as tc:
        with (
            tc.tile_pool(name="singles", bufs=1) as singles,
            tc.tile_pool(name="io_in", bufs=3) as io_in,
            tc.tile_pool(name="io_tg", bufs=3) as io_tg,
            tc.tile_pool(name="gp", bufs=2) as gp_pool,
            tc.tile_pool(name="small", bufs=1) as small,
        ):
            diag_raw = singles.tile([_P, n_tiles], f32)
            nc.sync.dma_start(out=diag_raw, in_=diag[:, :])
            diag_sb = singles.tile([_P, n_tiles], f32)
            nc.vector.tensor_copy(out=diag_sb, in_=diag_raw)
            negbig = singles.tile([_P, 1], f32)
            nc.vector.memset(negbig, _BIG_NEG)
            # top-8 per (row, tile), filled by the main loop
            vfin = singles.tile([_P, n_tiles, 8], f32)

            # tile 0 is processed in column chunks (DVE-only) so compute
            # starts as soon as the first chunk lands, instead of waiting
            # for the full 5 MiB tile-0 DMA
            chunk0 = ncols >= 4096 and ncols % 4 == 0
            vcol0 = None
            if chunk0:
                vcol0 = singles.tile([_P, 4, 8], f32)

            for t in range(n_tiles):
                rows = slice(t * _P, (t + 1) * _P)
                in_t = io_in.tile([_P, ncols], f32)
                tg_t = io_tg.tile([_P, ncols], tdt)
                if t == 0 and chunk0:
                    w = ncols // 4
                    for c in range(4):
                        cs = slice(c * w, (c + 1) * w)
                        nc.sync.dma_start(out=in_t[:, cs], in_=inp[rows, cs])
                        nc.sync.dma_start(out=tg_t[:, cs], in_=tgt[rows, cs])
                    for c in range(4):
                        cs = slice(c * w, (c + 1) * w)
                        nc.vector.scalar_tensor_tensor(
                            out=in_t[:, cs], in0=tg_t[:, cs], scalar=_BIG_NEG,
                            in1=in_t[:, cs],
                            op0=mybir.AluOpType.mult, op1=mybir.AluOpType.add)
                        nc.vector.max(out=vcol0[:, c, :], in_=in_t[:, cs])
                    nc.vector.max(out=vfin[:, 0, :], in_=vcol0[:, :, :])
                    continue
                nc.sync.dma_start(out=in_t, in_=inp[rows, :])
                nc.sync.dma_start(out=tg_t, in_=tgt[rows, :])
                # masked = (target * -1e9) + input, in place; the column
                # range is split between GPSIMD and DVE (see _GP_COLS),
                # with the GPSIMD share in two sub-chunks to shorten its
                # exclusive holds of the shared SBUF port
                gp_cols = min(_GP_COLS, ncols // 4)
                sp = ncols - gp_cols
                if gp_cols:
                    ug = gp_pool.tile([_P, gp_cols], f32)
                    nc.gpsimd.tensor_tensor(
                        out=ug, in0=tg_t[:, sp:],
                        in1=negbig.to_broadcast([_P, gp_cols]),
                        op=mybir.AluOpType.mult)
                    nc.gpsimd.tensor_tensor(
                        out=in_t[:, sp:], in0=in_t[:, sp:], in1=ug,
                        op=mybir.AluOpType.add)
                nc.vector.scalar_tensor_tensor(
                    out=in_t[:, :sp], in0=tg_t[:, :sp], scalar=_BIG_NEG,
                    in1=in_t[:, :sp],
                    op0=mybir.AluOpType.mult, op1=mybir.AluOpType.add)
                nc.vector.max(out=vfin[:, t, :], in_=in_t)

            # ---- vectorized epilogue over all tiles: [128, n_tiles, 3] ----
            sh = [_P, n_tiles, _K]
            v = small.tile(sh, f32)                    # top-3, descending
            nc.vector.tensor_copy(out=v, in_=vfin[:, :, 0:_K])
            p_b = diag_sb.unsqueeze(-1).to_broadcast(sh)

            x = small.tile(sh, f32)                    # x = sim_n - sim_p
            nc.vector.tensor_tensor(out=x, in0=v, in1=p_b,
                                    op=mybir.AluOpType.subtract)
            # a = clip(|x|, 0.1, 0.3)   (|x| as max(x, -x), bitwise exact)
            negx = small.tile(sh, f32)
            nc.vector.tensor_scalar(out=negx, in0=x, scalar1=-1.0,
                                    scalar2=None, op0=mybir.AluOpType.mult)
            a = small.tile(sh, f32)
            nc.vector.tensor_tensor(out=a, in0=x, in1=negx,
                                    op=mybir.AluOpType.max)
            nc.vector.tensor_scalar(out=a, in0=a, scalar1=0.1, scalar2=0.3,
                                    op0=mybir.AluOpType.max,
                                    op1=mybir.AluOpType.min)
            # loss = relu(x + a); active = (x + a) > 0
            xa = small.tile(sh, f32)
            nc.vector.tensor_tensor(out=xa, in0=x, in1=a,
                                    op=mybir.AluOpType.add)
            l = small.tile(sh, f32)
            nc.vector.tensor_scalar(out=l, in0=xa, scalar1=0.0, scalar2=None,
                                    op0=mybir.AluOpType.max)
            act = small.tile(sh, i32)
            nc.vector.tensor_scalar(out=act, in0=xa, scalar1=0.0, scalar2=None,
                                    op0=mybir.AluOpType.is_gt)
            # s = where(active, v, -50)
            s = small.tile(sh, f32)
            nc.vector.memset(s, _NEG_FILL)
            nc.vector.copy_predicated(out=s, mask=act, data=v)
            # softmax(s / tau) over K, with max-subtraction (matches jax)
            smax = small.tile([_P, n_tiles], f32)
            nc.vector.reduce_max(out=smax, in_=s, axis=mybir.AxisListType.X)
            s2 = small.tile(sh, f32)
            nc.vector.tensor_tensor(out=s2, in0=s,
                                    in1=smax.unsqueeze(-1).to_broadcast(sh),
                                    op=mybir.AluOpType.subtract)
            e = small.tile(sh, f32)
            nc.scalar.activation(out=e, in_=s2,
                                 func=mybir.ActivationFunctionType.Exp,
                                 scale=_INV_TAU)
            z = small.tile([_P, n_tiles], f32)
            nc.vector.reduce_sum(out=z, in_=e, axis=mybir.AxisListType.X)
            r = small.tile([_P, n_tiles], f32)
            nc.vector.reciprocal(out=r, in_=z)
            w = small.tile(sh, f32)
            nc.vector.tensor_tensor(out=w, in0=e,
                                    in1=r.unsqueeze(-1).to_broadcast(sh),
                                    op=mybir.AluOpType.mult)
            lw = small.tile(sh, f32)
            nc.vector.tensor_tensor(out=lw, in0=l, in1=w,
                                    op=mybir.AluOpType.mult)
            out_sb = small.tile([_P, n_tiles], f32)
            nc.vector.reduce_sum(out=out_sb, in_=lw, axis=mybir.AxisListType.X)
            nc.sync.dma_start(out=out[:, :], in_=out_sb)
    nc.compile()
    return nc


def _prepare_in_maps(inp: np.ndarray, tgt: np.ndarray, ncores: int):
    b, ncols = inp.shape
    rows = b // ncores
    n_tiles = rows // _P
    d = np.ascontiguousarray(np.diagonal(inp)).astype(np.float32, copy=False)
    if _TGT_INT8:
        # 0/1 int32 little-endian: byte 0 of each element carries the value
        tgt_v = np.ascontiguousarray(tgt.view(np.int8)[:, ::4])
    else:
        tgt_v = tgt
    in_maps = []
    for c in range(ncores):
        sl = slice(c * rows, (c + 1) * rows)
        diag_c = np.ascontiguousarray(d[sl].reshape(n_tiles, _P).T)
        in_maps.append({
            "inp": np.ascontiguousarray(inp[sl]),
            "tgt": np.ascontiguousarray(tgt_v[sl]),
            "diag": diag_c,
        })
    return in_maps


def kernel(input, target):
    inp = np.asarray(input, dtype=np.float32)
    tgt = np.asarray(target, dtype=np.int32)
    b, ncols = inp.shape

    nc = _build_nc(b // _NCORES, ncols)
    in_maps = _prepare_in_maps(inp, tgt, _NCORES)
    res = run_bass_kernel_spmd(nc, in_maps, list(range(_NCORES)))
    total = 0.0
    for r in res.results:
        total += r["out"].astype(np.float64).sum()
    return np.asarray(total / (b * _K), dtype=np.float32)


if __name__ == "__main__":
    rng = np.random.default_rng(0)
    b = _B
    x = rng.standard_normal((b, b), dtype=np.float32)
    t = rng.integers(0, 2, size=(b, b)).astype(np.int32)
    np.fill_diagonal(t, 1)
    print(kernel(x, t))


# revision 22
# speedup vs baseline: 1.2264x; 1.0012x over previous
"""Trainium2 Bass kernel for the ATripletMarginLossOHNMDM loss.

Per row i of an (B, B) input:
  sim_p      = input[i, i]
  masked     = where(target[i]==0, input[i], -1e9)
  sim_n[0:3] = top-3 values of masked          (hard negatives)
  d          = clip(|sim_p - sim_n|, 0.1, 0.3)
  loss       = relu(sim_n - sim_p + d)
  s          = where(loss>0, sim_n, -50)
  w          = softmax(s / 0.1)      (with max-subtraction, as jax.nn.softmax)
  out        = mean over (B, 3) of loss * w

Sharded by rows across 8 NeuronCores (1024 rows each). Per core, per
128-row tile:
  - DMA the input rows (f32) and the target rows. The target holds only
    0/1, so the host passes `target.view(int8)[:, ::4]` — a strided byte
    view of the original int32 buffer (pure data movement, no
    arithmetic) — which cuts target DMA traffic 4x.
  - one DVE scalar_tensor_tensor applies the mask in place:
        m = (target * -1e9) + input
  - one DVE Max8 instruction returns the top-8 per row -> top-3,
    collected into a [128, n_tiles, 8] buffer
A single vectorized epilogue then computes the margin/softmax math for
all tiles at once on [128, n_tiles, 3], and the per-(partition, tile)
partial sums are DMA'd out as [128, n_tiles]. The final mean over the
8 * 128 * n_tiles partials is computed on host.
"""

import numpy as np

import concourse.bacc as bacc
import concourse.mybir as mybir
import concourse.tile as tile
from concourse.bass_utils import run_bass_kernel_spmd

_B = 8192          # full problem size (rows == cols)
_NCORES = 8
_P = 128           # SBUF partitions
_K = 3
_BIG_NEG = -1.0e9  # mask fill; far below any real similarity
_NEG_FILL = -50.0  # reference's softmax mask fill (must match exactly)
_INV_TAU = 10.0    # 1 / 0.1
_TGT_INT8 = True   # pass target as int8 byte view (0/1 values, lossless)
# Columns of the mask pass offloaded to GPSIMD (as two tensor_tensor ops —
# the fused scalar_tensor_tensor opcode is not legal on Pool). Pool ops take
# the SBUF port pair shared with the DVE, so the chunk is sized to run inside
# the DVE Max8 window (Max8 only uses the DVE-dedicated port). 0 disables.
_GP_COLS = 2048


def _build_nc(rows_per_core: int, ncols: int) -> bacc.Bacc:
    n_tiles = rows_per_core // _P
    f32 = mybir.dt.float32
    i32 = mybir.dt.int32
    tdt = mybir.dt.int8 if _TGT_INT8 else i32

    nc = bacc.Bacc()
    inp = nc.dram_tensor("inp", [rows_per_core, ncols], f32,
                         kind="ExternalInput")
    tgt = nc.dram_tensor("tgt", [rows_per_core, ncols], tdt,
                         kind="ExternalInput")
    # diag[p, t] = input diagonal element of local row t*128 + p
    diag = nc.dram_tensor("diag", [_P, n_tiles], f32, kind="ExternalInput")
    out = nc.dram_tensor("out", [_P, n_tiles], f32, kind="ExternalOutput")

    with tile.TileContext(nc) as tc:
        with (
            tc.tile_pool(name="singles", bufs=1) as singles,
            tc.tile_pool(name="io_in", bufs=3) as io_in,
            tc.tile_pool(name="io_tg", bufs=3) as io_tg,
            tc.tile_pool(name="gp", bufs=2) as gp_pool,
            tc.tile_pool(name="small", bufs=1) as small,
        ):
            diag_raw = singles.tile([_P, n_tiles], f32)
            nc.sync.dma_start(out=diag_raw, in_=diag[:, :])
            diag_sb = singles.tile([_P, n_tiles], f32)
            nc.vector.tensor_copy(out=diag_sb, in_=diag_raw)
            negbig = singles.tile([_P, 1], f32)
            nc.vector.memset(negbig, _BIG_NEG)
            # top-8 per (row, tile), filled by the main loop
            vfin = singles.tile([_P, n_tiles, 8], f32)

            # tile 0 is processed in column chunks (DVE-only) so compute
            # starts as soon as the first chunk lands, instead of waiting
            # for the full 5 MiB tile-0 DMA
            chunk0 = ncols >= 4096 and ncols % 4 == 0
            vcol0 = None
            if chunk0:
                vcol0 = singles.tile([_P, 4, 8], f32)
            prev_stt = None

            for t in range(n_tiles):
                rows = slice(t * _P, (t + 1) * _P)
                in_t = io_in.tile([_P, ncols], f32)
                tg_t = io_tg.tile([_P, ncols], tdt)
                if t == 0 and chunk0:
                    w = ncols // 4
                    for c in range(4):
                        cs = slice(c * w, (c + 1) * w)
                        nc.sync.dma_start(out=in_t[:, cs], in_=inp[rows, cs])
                        nc.sync.dma_start(out=tg_t[:, cs], in_=tgt[rows, cs])
                    for c in range(4):
                        cs = slice(c * w, (c + 1) * w)
                        nc.vector.scalar_tensor_tensor(
                            out=in_t[:, cs], in0=tg_t[:, cs], scalar=_BIG_NEG,
                            in1=in_t[:, cs],
                            op0=mybir.AluOpType.mult, op1=mybir.AluOpType.add)
                        nc.vector.max(out=vcol0[:, c, :], in_=in_t[:, cs])
                    nc.vector.max(out=vfin[:, 0, :], in_=vcol0[:, :, :])
                    continue
                nc.sync.dma_start(out=in_t, in_=inp[rows, :])
                nc.sync.dma_start(out=tg_t, in_=tgt[rows, :])
                # masked = (target * -1e9) + input, in place; the column
                # range is split between GPSIMD and DVE (see _GP_COLS)
                gp_cols = min(_GP_COLS, ncols // 4)
                sp = ncols - gp_cols
                if gp_cols:
                    ug = gp_pool.tile([_P, gp_cols], f32)
                    gp_mul = nc.gpsimd.tensor_tensor(
                        out=ug, in0=tg_t[:, sp:],
                        in1=negbig.to_broadcast([_P, gp_cols]),
                        op=mybir.AluOpType.mult)
                    if prev_stt is not None:
                        # Pool ops take the SBUF port pair shared with the
                        # DVE; gate them behind the previous tile's DVE mask
                        # op so they run inside the Max8 window (Max8 only
                        # uses the DVE-dedicated port) instead of blocking
                        # the next DVE mask op mid-instruction.
                        tile.add_dep_helper(
                            gp_mul.ins, prev_stt.ins,
                            reason="phase GPSIMD into the Max8 port window")
                    nc.gpsimd.tensor_tensor(
                        out=in_t[:, sp:], in0=in_t[:, sp:], in1=ug,
                        op=mybir.AluOpType.add)
                prev_stt = nc.vector.scalar_tensor_tensor(
                    out=in_t[:, :sp], in0=tg_t[:, :sp], scalar=_BIG_NEG,
                    in1=in_t[:, :sp],
                    op0=mybir.AluOpType.mult, op1=mybir.AluOpType.add)
                nc.vector.max(out=vfin[:, t, :], in_=in_t)

            # ---- vectorized epilogue over all tiles: [128, n_tiles, 3] ----
            sh = [_P, n_tiles, _K]
            v = small.tile(sh, f32)                    # top-3, descending
            nc.vector.tensor_copy(out=v, in_=vfin[:, :, 0:_K])
            p_b = diag_sb.unsqueeze(-1).to_broadcast(sh)

            x = small.tile(sh, f32)                    # x = sim_n - sim_p
            nc.vector.tensor_tensor(out=x, in0=v, in1=p_b,
                                    op=mybir.AluOpType.subtract)
            # a = clip(|x|, 0.1, 0.3)   (|x| as max(x, -x), bitwise exact)
            negx = small.tile(sh, f32)
            nc.vector.tensor_scalar(out=negx, in0=x, scalar1=-1.0,
                                    scalar2=None, op0=mybir.AluOpType.mult)
            a = small.tile(sh, f32)
            nc.vector.tensor_tensor(out=a, in0=x, in1=negx,
                                    op=mybir.AluOpType.max)
            nc.vector.tensor_scalar(out=a, in0=a, scalar1=0.1, scalar2=0.3,
                                    op0=mybir.AluOpType.max,
                                    op1=mybir.AluOpType.min)
            # loss = relu(x + a); active = (x + a) > 0
            xa = small.tile(sh, f32)
            nc.vector.tensor_tensor(out=xa, in0=x, in1=a,
                                    op=mybir.AluOpType.add)
            l = small.tile(sh, f32)
            nc.vector.tensor_scalar(out=l, in0=xa, scalar1=0.0, scalar2=None,
                                    op0=mybir.AluOpType.max)
            act = small.tile(sh, i32)
            nc.vector.tensor_scalar(out=act, in0=xa, scalar1=0.0, scalar2=None,
                                    op0=mybir.AluOpType.is_gt)
            # s = where(active, v, -50)
            s = small.tile(sh, f32)
            nc.vector.memset(s, _NEG_FILL)
            nc.vector.copy_predicated(out=s, mask=act, data=v)
            # softmax(s / tau) over K, with max-subtraction (matches jax)
            smax = small.tile([_P, n_tiles], f32)
            nc.vector.reduce_max(out=smax, in_=s, axis=mybir.AxisListType.X)
            s2 = small.tile(sh, f32)
            nc.vector.tensor_tensor(out=s2, in0=s,
                                    in1=smax.unsqueeze(-1).to_broadcast(sh),
                                    op=mybir.AluOpType.subtract)
            e = small.tile(sh, f32)
            nc.scalar.activation(out=e, in_=s2,
                                 func=mybir.ActivationFunctionType.Exp,
                                 scale=_INV_TAU)
            z = small.tile([_P, n_tiles], f32)
            nc.vector.reduce_sum(out=z, in_=e, axis=mybir.AxisListType.X)
            r = small.tile([_P, n_tiles], f32)
            nc.vector.reciprocal(out=r, in_=z)
            w = small.tile(sh, f32)
            nc.vector.tensor_tensor(out=w, in0=e,
                                    in1=r.unsqueeze(-1).to_broadcast(sh),
                                    op=mybir.AluOpType.mult)
            lw = small.tile(sh, f32)
            nc.vector.tensor_tensor(out=lw, in0=l, in1=w,
                                    op=mybir.AluOpType.mult)
            out_sb = small.tile([_P, n_tiles], f32)
            nc.vector.reduce_sum(out=out_sb, in_=lw, axis=mybir.AxisListType.X)
            nc.sync.dma_start(out=out[:, :], in_=out_sb)
    nc.compile()
    return nc


def _prepare_in_maps(inp: np.ndarray, tgt: np.ndarray, ncores: int):
    b, ncols = inp.shape
    rows = b // ncores
    n_tiles = rows // _P
    d = np.ascontiguousarray(np.diagonal(inp)).astype(np.float32, copy=False)
    if _TGT_INT8:
        # 0/1 int32 little-endian: byte 0 of each element carries the value
        tgt_v = np.ascontiguousarray(tgt.view(np.int8)[:, ::4])
    else:
        tgt_v = tgt
    in_maps = []
    for c in range(ncores):
        sl = slice(c * rows, (c + 1) * rows)
        diag_c = np.ascontiguousarray(d[sl].reshape(n_tiles, _P).T)
        in_maps.append({
            "inp": np.ascontiguousarray(inp[sl]),
            "tgt": np.ascontiguousarray(tgt_v[sl]),
            "diag": diag_c,
        })
    return in_maps


_NC_CACHE = {}


def kernel(input, target):
    inp = np.asarray(input, dtype=np.float32)
    tgt = np.asarray(target, dtype=np.int32)
    b, ncols = inp.shape

    key = (b, ncols)
    nc = _NC_CACHE.get(key)
    if nc is None:
        nc = _NC_CACHE[key] = _build_nc(b // _NCORES, ncols)
    in_maps = _prepare_in_maps(inp, tgt, _NCORES)
    res = run_bass_kernel_spmd(nc, in_maps, list(range(_NCORES)))
    total = 0.0
    for r in res.results:
        total += r["out"].astype(np.float64).sum()
    return np.asarray(total / (b * _K), dtype=np.float32)


if __name__ == "__main__":
    rng = np.random.default_rng(0)
    b = _B
    x = rng.standard_normal((b, b), dtype=np.float32)
    t = rng.integers(0, 2, size=(b, b)).astype(np.int32)
    np.fill_diagonal(t, 1)
    print(kernel(x, t))
